# revision 9
# baseline (speedup 1.0000x reference)
"""Distributed GIN (5-layer) Bass kernel for one TRN2 chip (8 NeuronCores).

Strategy (dst-sharded message passing, replicated features):
- Each core owns 6250 dst nodes. Node features live replicated in HBM as
  fp16 rows [50176, 128] (node n at row (n//6250)*6272 + n%6250, cols 0:64).
- Per layer, each core gathers its in-edges' source rows with
  dma_gather(transpose=True) -> [128ch, slots] fp16 tiles, then does
  fixed-width windowed segment sums on the Vector engine. Window sizes are
  order statistics of per-core degree (identical across cores -> SPMD safe).
- The int16 gather-index limit (32767) forces two gather "sides": rows
  [0, 17408) and [17408, 50176). Each side reduces in its own degree-sorted
  order into a per-side scratch [rows,128] fp16 in HBM; two tiny follow-up
  gathers (g3) realign both partial aggregates to canonical column order.
- MLP: PE matmuls (64x64 weights), BatchNorm via a [64,2] AllReduce of
  sum/sumsq (+corrections for the 22 fake columns), fused scale+bias+ReLU
  on the Scalar engine. New features are cast fp16, transposed to row-major
  via PE, AllGathered (compact [6272,64]) and expanded into the row buffer.
- Final layer: global_add_pool via one-hot matmuls against the fp16
  staging, [128,64] AllReduce, tiny MLP + log_softmax on every core.
"""
import sys

sys.path.insert(0, "/opt/trn_rl_repo")

import numpy as np

N = 50000
E = 800000
NC = 8
OWN = N // NC            # 6250
COLS = 6272              # 49 * 128
NCH = COLS // 128        # 49
TOTROWS = NC * COLS      # 50176
R = 17408                # side split: rows < R -> side a, else side b
GSLOT = 8192
DIM = 64
NGRAPH = 128
NLAYER = 5
BN_EPS = 1e-5

PAD_ROW_A = OWN                     # core 0 fake row (zero), < R
PAD_ROW_B = 7 * COLS + OWN          # 50154, core 7 fake row (zero), >= R


# ---------------------------------------------------------------------------
# host-side preprocessing
# ---------------------------------------------------------------------------

class SideStruct:
    """SPMD-uniform gather/reduce structure for one side."""

    def __init__(self, per_core_rows, base, side_name):
        self.base = base
        self.name = side_name
        deg = np.zeros((NC, OWN), np.int32)
        for c in range(NC):
            for v, rr in per_core_rows[c].items():
                deg[c, v] = len(rr)
        deg = np.maximum(deg, 1)
        self.order = np.argsort(-deg, axis=1, kind="stable")
        sorted_deg = -np.sort(-deg, axis=1)
        W = sorted_deg.max(axis=0)             # [OWN] monotone non-increasing
        self.W = W
        self.buckets, self.counts = [], []
        i = 0
        while i < OWN:
            j = i
            while j < OWN and W[j] == W[i]:
                j += 1
            self.buckets.append(int(W[i]))
            self.counts.append(j - i)
            i = j
        self.cols = OWN
        self.scr_w = ((self.cols + 127) // 128) * 128      # 6272
        self.zrow = self.scr_w
        self.scr_rows = self.scr_w + 16

        # chunked slot layout: runs[chunk] = [(slot_off, col_off, nwin, D)]
        self.runs = []
        cur, slot, col = [], 0, 0
        self.nslots = 0
        for b, cnt in zip(self.buckets, self.counts):
            left = cnt
            while left > 0:
                fit = (GSLOT - slot) // b
                if fit == 0:
                    self.runs.append(cur)
                    self.nslots += GSLOT
                    cur, slot = [], 0
                    continue
                take = min(left, fit)
                cur.append((slot, col, take, b))
                slot += take * b
                col += take
                left -= take
        if cur or not self.runs:
            self.runs.append(cur)
            self.nslots += GSLOT
        self.nchunks = len(self.runs)
        assert col == self.cols

        pad_row = PAD_ROW_A if side_name == "a" else PAD_ROW_B
        self.pi_pos = np.zeros((NC, OWN), np.int32)
        self.idx = np.zeros((NC, self.nchunks * GSLOT), np.int16)
        for c in range(NC):
            win_rows = []
            for p in range(OWN):
                v = int(self.order[c, p])
                self.pi_pos[c, v] = p
                win_rows.append((per_core_rows[c].get(v, []), int(self.W[p])))
            w = 0
            flat = np.zeros(self.nchunks * GSLOT, np.int64)
            for ch, runs in enumerate(self.runs):
                base_slot = ch * GSLOT
                for (slot_off, col_off, nwin, D) in runs:
                    for k in range(nwin):
                        rows, DD = win_rows[w]
                        assert DD == D
                        s0 = base_slot + slot_off + k * D
                        nr = len(rows)
                        flat[s0:s0 + nr] = rows
                        flat[s0 + nr:s0 + D] = pad_row
                        w += 1
            assert w == self.cols
            rel = np.clip(flat - self.base, 0, None)
            assert rel.max() <= 32767
            self.idx[c] = rel.astype(np.int16)

        self.g3 = np.full((NC, COLS), self.zrow, np.int32)
        for c in range(NC):
            self.g3[c, :OWN] = self.pi_pos[c]


def wrap_idx(a):
    """flat slot order -> [128, S//16] dma_gather index layout."""
    S = len(a)
    assert S % 16 == 0
    w = a.reshape(S // 16, 16).T
    return np.ascontiguousarray(np.tile(w, (8, 1)))


def wrap_idx_chunked(a, nchunks):
    return np.hstack([wrap_idx(a[c * GSLOT:(c + 1) * GSLOT]) for c in range(nchunks)])


def preprocess(x, edge_index, batch, params):
    x = np.asarray(x, np.float32)
    src = np.asarray(edge_index[0], np.int64)
    dst = np.asarray(edge_index[1], np.int64)
    batch = np.asarray(batch, np.int64)

    srow = (src // OWN) * COLS + (src % OWN)
    dcore = dst // OWN
    dloc = dst % OWN

    rows_a = [dict() for _ in range(NC)]
    rows_b = [dict() for _ in range(NC)]
    order = np.lexsort((srow, dst))
    for e in order:
        c = int(dcore[e])
        v = int(dloc[e])
        r = int(srow[e])
        tgt = rows_a[c] if r < R else rows_b[c]
        tgt.setdefault(v, []).append(r)
    for n in range(N):
        c, v = n // OWN, n % OWN
        r = (n // OWN) * COLS + (n % OWN)
        tgt = rows_a[c] if r < R else rows_b[c]
        tgt.setdefault(v, []).append(r)

    sa = SideStruct(rows_a, 0, "a")
    sb = SideStruct(rows_b, R, "b")

    h0 = np.zeros((TOTROWS, 128), np.float16)
    for c in range(NC):
        h0[c * COLS:c * COLS + OWN, :DIM] = \
            x[c * OWN:(c + 1) * OWN].astype(np.float16)

    Gp = np.zeros((NC, COLS, NGRAPH), np.float16)
    for c in range(NC):
        Gp[c, np.arange(OWN), batch[c * OWN:(c + 1) * OWN]] = 1.0

    convs = params["convs"]
    W1 = np.stack([np.asarray(p["W1"], np.float32) for p in convs])
    W2 = np.stack([np.asarray(p["W2"], np.float32) for p in convs])
    B1 = np.stack([np.asarray(p["b1"], np.float32) for p in convs])
    GM = np.stack([np.asarray(p["gamma"], np.float32) for p in convs])
    BT = np.stack([np.asarray(p["beta"], np.float32) for p in convs])
    B2 = np.stack([np.asarray(p["b2"], np.float32) for p in convs])
    nfake = COLS - OWN
    # PARAMS[l] is [64, 6] ch-major: b1, gamma, beta, b2, corr1, corr2
    PAR = np.stack([np.stack([B1[l], GM[l], BT[l], B2[l],
                              -nfake * B1[l], -nfake * B1[l] * B1[l]], axis=1)
                    for l in range(NLAYER)])                 # [5, 64, 6]
    LINB = np.zeros((64, 2), np.float32)
    LINB[:, 0] = np.asarray(params["lin1_b"], np.float32)
    LINB[:10, 1] = np.asarray(params["lin2_b"], np.float32)

    return dict(
        sa=sa, sb=sb, h0=h0, Gp=Gp, W1=W1, W2=W2, PAR=PAR,
        lin1_W=np.asarray(params["lin1_W"], np.float32),
        lin2_W=np.asarray(params["lin2_W"], np.float32),
        LINB=LINB, IDN=np.eye(128, dtype=np.float32),
    )


# ---------------------------------------------------------------------------
# bass kernel builder
# ---------------------------------------------------------------------------

def build_kernel(P):
    import concourse.bass as bass
    import concourse.bacc as bacc
    import concourse.tile as tile
    import concourse.mybir as mybir
    from concourse import library_config

    dt = mybir.dt
    AF = mybir.ActivationFunctionType
    ALU = mybir.AluOpType
    AX = mybir.AxisListType
    sa, sb = P["sa"], P["sb"]

    nc = bacc.Bacc("TRN2", target_bir_lowering=False, debug=False,
                   enable_asserts=True, num_devices=NC)

    # ---- I/O -----------------------------------------------------------
    h0_d = nc.dram_tensor("h0", [TOTROWS, 128], dt.float16, kind="ExternalInput")
    ixa_d = nc.dram_tensor("idxa", [128, sa.nchunks * (GSLOT // 16)], dt.int16,
                           kind="ExternalInput")
    ixb_d = nc.dram_tensor("idxb", [128, sb.nchunks * (GSLOT // 16)], dt.int16,
                           kind="ExternalInput")
    g3a_d = nc.dram_tensor("g3a", [128, COLS // 16], dt.int16, kind="ExternalInput")
    g3b_d = nc.dram_tensor("g3b", [128, COLS // 16], dt.int16, kind="ExternalInput")
    gp_d = nc.dram_tensor("gpool", [COLS, NGRAPH], dt.float16, kind="ExternalInput")
    w1_d = nc.dram_tensor("w1", [NLAYER, 64, 64], dt.float32, kind="ExternalInput")
    w2_d = nc.dram_tensor("w2", [NLAYER, 64, 64], dt.float32, kind="ExternalInput")
    par_d = nc.dram_tensor("par", [NLAYER, 64, 6], dt.float32, kind="ExternalInput")
    l1w_d = nc.dram_tensor("l1w", [64, 64], dt.float32, kind="ExternalInput")
    l2w_d = nc.dram_tensor("l2w", [64, 10], dt.float32, kind="ExternalInput")
    lb_d = nc.dram_tensor("lb", [64, 2], dt.float32, kind="ExternalInput")
    idn_d = nc.dram_tensor("idn", [128, 128], dt.float32, kind="ExternalInput")
    out_d = nc.dram_tensor("out", [NGRAPH, 10], dt.float32, kind="ExternalOutput")

    # ---- internal DRAM -------------------------------------------------
    hstore = nc.dram_tensor("hstore", [TOTROWS, 128], dt.float16, kind="Internal")
    scr_a = nc.dram_tensor("scr_a", [sa.scr_rows, 128], dt.float16, kind="Internal")
    scr_b = nc.dram_tensor("scr_b", [sb.scr_rows, 128], dt.float16, kind="Internal")
    contrib = nc.dram_tensor("contrib", [COLS, 64], dt.float16, kind="Internal")
    agout = nc.dram_tensor("agout", [TOTROWS, 64], dt.float16, kind="Internal",
                           addr_space="Shared")
    st_in = nc.dram_tensor("st_in", [64, 2], dt.float32, kind="Internal")
    st_out = nc.dram_tensor("st_out", [64, 2], dt.float32, kind="Internal",
                            addr_space="Shared")
    pl_in = nc.dram_tensor("pl_in", [128, 64], dt.float32, kind="Internal")
    pl_out = nc.dram_tensor("pl_out", [128, 64], dt.float32, kind="Internal",
                            addr_space="Shared")

    RG = [list(range(NC))]

    with tile.TileContext(nc) as tc:
        nc.gpsimd.load_library(library_config.mlp)
        with tc.tile_pool(name="const", bufs=1) as constp, \
             tc.tile_pool(name="idxp", bufs=1) as idxp, \
             tc.tile_pool(name="gbuf", bufs=2) as gbufp, \
             tc.tile_pool(name="agg", bufs=1) as aggp, \
             tc.tile_pool(name="big", bufs=1) as bigp, \
             tc.tile_pool(name="stage", bufs=1) as stagep, \
             tc.tile_pool(name="wt", bufs=1) as wtp, \
             tc.tile_pool(name="mm", bufs=4, space="PSUM") as mmp, \
             tc.tile_pool(name="tp", bufs=2, space="PSUM") as tpp, \
             tc.tile_pool(name="fin", bufs=1, space="PSUM") as finp:

            # ---- one-time loads ----
            idn = constp.tile([128, 128], dt.float32, tag="idn")
            nc.sync.dma_start(idn[:], idn_d[:])
            cpack = constp.tile([64, 80], dt.float32, tag="cpack")
            nc.sync.dma_start(cpack[:, 0:64], l1w_d[:])
            nc.sync.dma_start(cpack[:, 64:74], l2w_d[:])
            nc.sync.dma_start(cpack[:, 74:76], lb_d[:])
            ixa = idxp.tile([128, sa.nchunks * (GSLOT // 16)], dt.int16, tag="ixa")
            ixb = idxp.tile([128, sb.nchunks * (GSLOT // 16)], dt.int16, tag="ixb")
            g3a = idxp.tile([128, COLS // 16], dt.int16, tag="g3a")
            g3b = idxp.tile([128, COLS // 16], dt.int16, tag="g3b")
            nc.sync.dma_start(ixa[:], ixa_d[:])
            nc.sync.dma_start(ixb[:], ixb_d[:])
            nc.sync.dma_start(g3a[:], g3a_d[:])
            nc.sync.dma_start(g3b[:], g3b_d[:])

            # zero rows of the scratch tensors
            zr = constp.tile([1, 128], dt.float16, tag="zr")
            nc.vector.memset(zr[:], 0.0)
            nc.sync.dma_start(scr_a[sa.zrow:sa.zrow + 1, :], zr[:])
            nc.sync.dma_start(scr_b[sb.zrow:sb.zrow + 1, :], zr[:])

            staging = None

            def side_pass(side, idx_tile, g3_tile, scr_d, src_rows):
                """gather+reduce+transpose+scratch+g3 for one side."""
                agg = aggp.tile([64, OWN], dt.float32, tag="agg")
                for ch in range(side.nchunks):
                    buf = gbufp.tile([128, 1, GSLOT], dt.float16, tag="gbuf")
                    nc.gpsimd.dma_gather(
                        buf[:], src_rows, idx_tile[:, ch * 512:(ch + 1) * 512],
                        GSLOT, GSLOT, 128, transpose=True,
                        single_packet=False)
                    for (so, co, nwin, D) in side.runs[ch]:
                        win = buf[0:64, 0, so:so + nwin * D] \
                            .rearrange("p (n d) -> p n d", d=D)
                        nc.vector.tensor_reduce(
                            agg[:, co:co + nwin], win, axis=AX.X, op=ALU.add)
                sta = stagep.tile([128, NCH * 64], dt.float16, tag="scrsta")
                for t in range(NCH):
                    w = 128 if t < NCH - 1 else OWN - 128 * (NCH - 1)
                    ps = tpp.tile([128, 64], dt.float32, tag="tp")
                    nc.tensor.transpose(ps[0:w, :], agg[:, t * 128:t * 128 + w],
                                        idn[0:64, 0:64])
                    if w < 128:
                        nc.vector.memset(sta[:, t * 64:(t + 1) * 64], 0.0)
                    nc.scalar.copy(sta[0:w, t * 64:(t + 1) * 64], ps[0:w, :])
                dst = scr_d[0:side.scr_w, 0:64].rearrange(
                    "(t p) c -> p t c", p=128)
                nc.sync.dma_start(dst, sta[:].rearrange("p (t c) -> p t c", c=64))
                g3buf = gbufp.tile([128, 1, GSLOT], dt.float16, tag="gbuf")
                nc.gpsimd.dma_gather(
                    g3buf[0:128, 0:1, 0:COLS], scr_d[:], g3_tile[:],
                    COLS, COLS, 128, transpose=True, single_packet=False)
                return g3buf

            for l in range(NLAYER):
                hsrc = h0_d if l == 0 else hstore
                # consume each side's g3 buffer immediately: it lives in the
                # rotating gbuf pool, so a delayed read would deadlock the
                # next side's gathers on its slot.
                ga = side_pass(sa, ixa, g3a, scr_a, hsrc[0:R, :])
                u = bigp.tile([64, COLS], dt.float32, tag="u")
                nc.vector.tensor_copy(u[:], ga[0:64, 0, 0:COLS])
                gb = side_pass(sb, ixb, g3b, scr_b, hsrc[R:R + 32768, :])
                z = bigp.tile([64, COLS], dt.float32, tag="z")
                nc.vector.tensor_copy(z[:], gb[0:64, 0, 0:COLS])
                nc.vector.tensor_tensor(u[:], u[:], z[:], op=ALU.add)

                # weights / params for this layer
                wts = wtp.tile([64, 144], dt.float32, tag="wts")
                w1, w2, par = wts[:, 0:64], wts[:, 64:128], wts[:, 128:134]
                nc.sync.dma_start(w1, w1_d[l])
                nc.sync.dma_start(w2, w2_d[l])
                nc.sync.dma_start(par, par_d[l])

                # z = u @ W1 + b1
                z = bigp.tile([64, COLS], dt.float32, tag="z")
                for k in range(0, COLS, 512):
                    n = min(512, COLS - k)
                    ps = mmp.tile([64, 512], dt.float32, tag="mm")
                    nc.tensor.matmul(ps[:, 0:n], w1, u[:, k:k + n],
                                     start=True, stop=True)
                    nc.scalar.activation(z[:, k:k + n], ps[:, 0:n],
                                         AF.Identity, bias=par[:, 0:1])

                # BN stats (+fake-col corrections), cross-core AllReduce
                bnv = wtp.tile([64, 16], dt.float32, tag="bnv")
                s1, s2 = bnv[:, 0:1], bnv[:, 1:2]
                stp, stg = bnv[:, 2:4], bnv[:, 4:6]
                mu, var = bnv[:, 6:7], bnv[:, 7:8]
                aven, bvec = bnv[:, 8:9], bnv[:, 9:10]
                sd, rsd = bnv[:, 10:11], bnv[:, 11:12]
                nc.vector.tensor_reduce(s1, z[:], axis=AX.X, op=ALU.add)
                zsq = bigp.tile([64, COLS], dt.float32, tag="u")
                nc.scalar.activation(zsq[:], z[:], AF.Square, accum_out=s2)
                nc.vector.tensor_tensor(stp[:, 0:1], s1, par[:, 4:5], op=ALU.add)
                nc.vector.tensor_tensor(stp[:, 1:2], s2, par[:, 5:6], op=ALU.add)
                nc.sync.dma_start(st_in[:], stp)
                nc.gpsimd.collective_compute(
                    "AllReduce", ALU.add, replica_groups=RG,
                    ins=[st_in[:].opt()], outs=[st_out[:].opt()])
                nc.sync.dma_start(stg, st_out[:])

                # avec = gamma * rsqrt(var+eps), bvec = beta - mu*avec
                nc.vector.tensor_scalar_mul(mu, stg[:, 0:1], 1.0 / N)
                nc.vector.tensor_scalar_mul(var, stg[:, 1:2], 1.0 / N)
                nc.vector.tensor_tensor(aven, mu, mu, op=ALU.mult)
                nc.vector.tensor_tensor(var, var, aven, op=ALU.subtract)
                nc.vector.tensor_scalar_add(var, var, BN_EPS)
                nc.scalar.activation(sd, var, AF.Sqrt)
                nc.vector.reciprocal(rsd, sd)
                nc.vector.tensor_tensor(aven, par[:, 1:2], rsd, op=ALU.mult)
                nc.vector.tensor_tensor(bvec, mu, aven, op=ALU.mult)
                nc.vector.tensor_tensor(bvec, par[:, 2:3], bvec, op=ALU.subtract)

                # zb = relu(avec*z + bvec); hp = relu(zb @ W2 + b2)
                zb = bigp.tile([64, COLS], dt.float32, tag="u")
                nc.scalar.activation(zb[:], z[:], AF.Relu, bias=bvec, scale=aven)
                hp = bigp.tile([64, COLS], dt.float32, tag="z")
                for k in range(0, COLS, 512):
                    n = min(512, COLS - k)
                    ps = mmp.tile([64, 512], dt.float32, tag="mm")
                    nc.tensor.matmul(ps[:, 0:n], w2, zb[:, k:k + n],
                                     start=True, stop=True)
                    nc.scalar.activation(hp[:, k:k + n], ps[:, 0:n],
                                         AF.Relu, bias=par[:, 3:4])

                # staging: transpose+cast to fp16 rows; zero fake rows
                staging = stagep.tile([128, NCH * 64], dt.float16, tag="hsta")
                lastw = OWN - 128 * (NCH - 1)       # 106 real rows in chunk 48
                for t in range(NCH):
                    ps = tpp.tile([128, 64], dt.float32, tag="tp")
                    nc.tensor.transpose(ps[:], hp[:, t * 128:(t + 1) * 128],
                                        idn[0:64, 0:64])
                    if t < NCH - 1:
                        nc.scalar.copy(staging[:, t * 64:(t + 1) * 64], ps[:])
                    else:
                        nc.vector.memset(staging[:, t * 64:(t + 1) * 64], 0.0)
                        nc.scalar.copy(staging[0:lastw, t * 64:(t + 1) * 64],
                                       ps[0:lastw, :])

                if l < NLAYER - 1:
                    nc.sync.dma_start(
                        contrib[:].rearrange("(t p) c -> p t c", p=128),
                        staging[:].rearrange("p (t c) -> p t c", c=64))
                    nc.gpsimd.collective_compute(
                        "AllGather", ALU.bypass, replica_groups=RG,
                        ins=[contrib[:].opt()], outs=[agout[:].opt()])
                    nc.sync.dma_start(hstore[:, 0:64], agout[:])

            # ---- pooling + final MLP ----
            plps = finp.tile([128, 64], dt.float32, tag="plps")
            for t in range(NCH):
                gt = gbufp.tile([128, NGRAPH], dt.float16, tag="gt")
                nc.sync.dma_start(gt[:], gp_d[t * 128:(t + 1) * 128, :])
                nc.tensor.matmul(plps[:], gt[:],
                                 staging[:, t * 64:(t + 1) * 64],
                                 start=(t == 0), stop=(t == NCH - 1))
            fin = wtp.tile([128, 560], dt.float32, tag="fin")
            pls, plg = fin[:, 0:64], fin[:, 64:128]
            pcm, z1 = fin[0:64, 128:256], fin[0:64, 256:384]
            lg = fin[0:10, 384:512]
            lt, ex, res = fin[:, 512:522], fin[:, 522:532], fin[:, 537:547]
            mx, mxn = fin[:, 532:533], fin[:, 533:534]
            se, ln, lnn = fin[:, 534:535], fin[:, 535:536], fin[:, 536:537]
            nc.vector.tensor_copy(pls, plps[:])
            nc.sync.dma_start(pl_in[:], pls)
            nc.gpsimd.collective_compute(
                "AllReduce", ALU.add, replica_groups=RG,
                ins=[pl_in[:].opt()], outs=[pl_out[:].opt()])
            nc.sync.dma_start(plg, pl_out[:])
            pcmp = finp.tile([64, 128], dt.float32, tag="pcmp")
            nc.tensor.transpose(pcmp[:], plg, idn[:])
            nc.vector.tensor_copy(pcm, pcmp[:])

            z1p = finp.tile([64, 128], dt.float32, tag="pcmp")
            nc.tensor.matmul(z1p[:], cpack[:, 0:64], pcm, start=True, stop=True)
            nc.scalar.activation(z1, z1p[:], AF.Relu, bias=cpack[:, 74:75])
            lgp = finp.tile([10, 128], dt.float32, tag="pcmp")
            nc.tensor.matmul(lgp[:], cpack[:, 64:74], z1, start=True, stop=True)
            nc.scalar.activation(lg, lgp[:], AF.Identity, bias=cpack[0:10, 75:76])
            ltp = finp.tile([128, 10], dt.float32, tag="pcmp")
            nc.tensor.transpose(ltp[:], lg, idn[0:10, 0:10])
            nc.vector.tensor_copy(lt, ltp[:])

            # log_softmax rows
            nc.vector.tensor_reduce(mx, lt, axis=AX.X, op=ALU.max)
            nc.vector.tensor_scalar_mul(mxn, mx, -1.0)
            nc.scalar.activation(ex, lt, AF.Exp, bias=mxn, accum_out=se)
            nc.scalar.activation(ln, se, AF.Ln)
            nc.vector.tensor_scalar_mul(lnn, ln, -1.0)
            nc.vector.tensor_scalar_add(res, lt, mxn)
            nc.vector.tensor_scalar_add(res, res, lnn)
            nc.sync.dma_start(out_d[:], res)

    nc.compile()
    return nc


# ---------------------------------------------------------------------------
# entry point
# ---------------------------------------------------------------------------

def make_in_maps(P):
    sa, sb = P["sa"], P["sb"]
    maps = []
    for c in range(NC):
        maps.append({
            "h0": P["h0"],
            "idxa": wrap_idx_chunked(sa.idx[c], sa.nchunks),
            "idxb": wrap_idx_chunked(sb.idx[c], sb.nchunks),
            "g3a": wrap_idx(sa.g3[c].astype(np.int16)),
            "g3b": wrap_idx(sb.g3[c].astype(np.int16)),
            "gpool": P["Gp"][c],
            "w1": P["W1"], "w2": P["W2"], "par": P["PAR"],
            "l1w": P["lin1_W"], "l2w": P["lin2_W"], "lb": P["LINB"],
            "idn": P["IDN"],
        })
    return maps


def kernel(x, edge_index, batch, params):
    from concourse import bass_utils

    P = preprocess(x, edge_index, batch, params)
    nc = build_kernel(P)
    in_maps = make_in_maps(P)
    res = bass_utils.run_bass_kernel_spmd(nc, in_maps, core_ids=list(range(NC)))
    return res.results[0]["out"].astype(np.float32)


# revision 10
# speedup vs baseline: 1.3372x; 1.3372x over previous
"""Distributed GIN (5-layer) Bass kernel for one TRN2 chip (8 NeuronCores).

Strategy (dst-sharded message passing, replicated features):
- Each core owns 6250 dst nodes. Node features live replicated in HBM as
  fp16 rows [50176, 128] (node n at row (n//6250)*6272 + n%6250, cols 0:64).
- Per layer, each core gathers its in-edges' source rows with
  dma_gather(transpose=True) -> [128ch, slots] fp16 tiles, then does
  fixed-width windowed segment sums on the Vector engine. Window sizes are
  order statistics of per-core degree (identical across cores -> SPMD safe).
- The int16 gather-index limit (32767) forces two gather "sides": rows
  [0, 17408) and [17408, 50176). Each side reduces in its own degree-sorted
  order into a per-side scratch [rows,128] fp16 in HBM; two tiny follow-up
  gathers (g3) realign both partial aggregates to canonical column order.
- MLP: PE matmuls (64x64 weights), BatchNorm via a [64,2] AllReduce of
  sum/sumsq (+corrections for the 22 fake columns), fused scale+bias+ReLU
  on the Scalar engine. New features are cast fp16, transposed to row-major
  via PE, AllGathered (compact [6272,64]) and expanded into the row buffer.
- Final layer: global_add_pool via one-hot matmuls against the fp16
  staging, [128,64] AllReduce, tiny MLP + log_softmax on every core.
"""
import sys

sys.path.insert(0, "/opt/trn_rl_repo")

import numpy as np

N = 50000
E = 800000
NC = 8
OWN = N // NC            # 6250
COLS = 6272              # 49 * 128
NCH = COLS // 128        # 49
TOTROWS = NC * COLS      # 50176
R = 17408                # side split: rows < R -> side a, else side b
GSLOT = 8192
DIM = 64
NGRAPH = 128
NLAYER = 5
BN_EPS = 1e-5

PAD_ROW_A = OWN                     # core 0 fake row (zero), < R
PAD_ROW_B = 7 * COLS + OWN          # 50154, core 7 fake row (zero), >= R


# ---------------------------------------------------------------------------
# host-side preprocessing
# ---------------------------------------------------------------------------

class SideStruct:
    """SPMD-uniform gather/reduce structure for one side."""

    def __init__(self, per_core_rows, base, side_name):
        self.base = base
        self.name = side_name
        deg = np.zeros((NC, OWN), np.int32)
        for c in range(NC):
            for v, rr in per_core_rows[c].items():
                deg[c, v] = len(rr)
        deg = np.maximum(deg, 1)
        self.order = np.argsort(-deg, axis=1, kind="stable")
        sorted_deg = -np.sort(-deg, axis=1)
        W = sorted_deg.max(axis=0)             # [OWN] monotone non-increasing
        self.W = W
        self.buckets, self.counts = [], []
        i = 0
        while i < OWN:
            j = i
            while j < OWN and W[j] == W[i]:
                j += 1
            self.buckets.append(int(W[i]))
            self.counts.append(j - i)
            i = j
        self.cols = OWN
        self.scr_w = ((self.cols + 127) // 128) * 128      # 6272
        self.zrow = self.scr_w
        self.scr_rows = self.scr_w + 16

        # chunked slot layout: runs[chunk] = [(slot_off, col_off, nwin, D)]
        self.runs = []
        cur, slot, col = [], 0, 0
        self.nslots = 0
        for b, cnt in zip(self.buckets, self.counts):
            left = cnt
            while left > 0:
                fit = (GSLOT - slot) // b
                if fit == 0:
                    self.runs.append(cur)
                    self.nslots += GSLOT
                    cur, slot = [], 0
                    continue
                take = min(left, fit)
                cur.append((slot, col, take, b))
                slot += take * b
                col += take
                left -= take
        if cur or not self.runs:
            self.runs.append(cur)
            self.nslots += GSLOT
        self.nchunks = len(self.runs)
        assert col == self.cols

        pad_row = PAD_ROW_A if side_name == "a" else PAD_ROW_B
        self.pi_pos = np.zeros((NC, OWN), np.int32)
        self.idx = np.zeros((NC, self.nchunks * GSLOT), np.int16)
        for c in range(NC):
            win_rows = []
            for p in range(OWN):
                v = int(self.order[c, p])
                self.pi_pos[c, v] = p
                win_rows.append((per_core_rows[c].get(v, []), int(self.W[p])))
            w = 0
            flat = np.zeros(self.nchunks * GSLOT, np.int64)
            for ch, runs in enumerate(self.runs):
                base_slot = ch * GSLOT
                for (slot_off, col_off, nwin, D) in runs:
                    for k in range(nwin):
                        rows, DD = win_rows[w]
                        assert DD == D
                        s0 = base_slot + slot_off + k * D
                        nr = len(rows)
                        flat[s0:s0 + nr] = rows
                        flat[s0 + nr:s0 + D] = pad_row
                        w += 1
            assert w == self.cols
            rel = np.clip(flat - self.base, 0, None)
            assert rel.max() <= 32767
            self.idx[c] = rel.astype(np.int16)

        self.g3 = np.full((NC, COLS), self.zrow, np.int32)
        for c in range(NC):
            self.g3[c, :OWN] = self.pi_pos[c]


def wrap_idx(a):
    """flat slot order -> [128, S//16] dma_gather index layout."""
    S = len(a)
    assert S % 16 == 0
    w = a.reshape(S // 16, 16).T
    return np.ascontiguousarray(np.tile(w, (8, 1)))


def wrap_idx_chunked(a, nchunks):
    return np.hstack([wrap_idx(a[c * GSLOT:(c + 1) * GSLOT]) for c in range(nchunks)])


def preprocess(x, edge_index, batch, params):
    x = np.asarray(x, np.float32)
    src = np.asarray(edge_index[0], np.int64)
    dst = np.asarray(edge_index[1], np.int64)
    batch = np.asarray(batch, np.int64)

    srow = (src // OWN) * COLS + (src % OWN)
    dcore = dst // OWN
    dloc = dst % OWN

    rows_a = [dict() for _ in range(NC)]
    rows_b = [dict() for _ in range(NC)]
    order = np.lexsort((srow, dst))
    for e in order:
        c = int(dcore[e])
        v = int(dloc[e])
        r = int(srow[e])
        tgt = rows_a[c] if r < R else rows_b[c]
        tgt.setdefault(v, []).append(r)
    for n in range(N):
        c, v = n // OWN, n % OWN
        r = (n // OWN) * COLS + (n % OWN)
        tgt = rows_a[c] if r < R else rows_b[c]
        tgt.setdefault(v, []).append(r)

    sa = SideStruct(rows_a, 0, "a")
    sb = SideStruct(rows_b, R, "b")

    h0 = np.zeros((TOTROWS, 128), np.float16)
    for c in range(NC):
        h0[c * COLS:c * COLS + OWN, :DIM] = \
            x[c * OWN:(c + 1) * OWN].astype(np.float16)

    Gp = np.zeros((NC, COLS, NGRAPH), np.float16)
    for c in range(NC):
        Gp[c, np.arange(OWN), batch[c * OWN:(c + 1) * OWN]] = 1.0

    convs = params["convs"]
    W1 = np.stack([np.asarray(p["W1"], np.float32) for p in convs])
    W2 = np.stack([np.asarray(p["W2"], np.float32) for p in convs])
    B1 = np.stack([np.asarray(p["b1"], np.float32) for p in convs])
    GM = np.stack([np.asarray(p["gamma"], np.float32) for p in convs])
    BT = np.stack([np.asarray(p["beta"], np.float32) for p in convs])
    B2 = np.stack([np.asarray(p["b2"], np.float32) for p in convs])
    nfake = COLS - OWN
    # PARAMS[l] is [64, 6] ch-major: b1, gamma, beta, b2, corr1, corr2
    PAR = np.stack([np.stack([B1[l], GM[l], BT[l], B2[l],
                              -nfake * B1[l], -nfake * B1[l] * B1[l]], axis=1)
                    for l in range(NLAYER)])                 # [5, 64, 6]
    LINB = np.zeros((64, 2), np.float32)
    LINB[:, 0] = np.asarray(params["lin1_b"], np.float32)
    LINB[:10, 1] = np.asarray(params["lin2_b"], np.float32)

    return dict(
        sa=sa, sb=sb, h0=h0, Gp=Gp, W1=W1, W2=W2, PAR=PAR,
        lin1_W=np.asarray(params["lin1_W"], np.float32),
        lin2_W=np.asarray(params["lin2_W"], np.float32),
        LINB=LINB, IDN=np.eye(128, dtype=np.float32),
    )


# ---------------------------------------------------------------------------
# bass kernel builder
# ---------------------------------------------------------------------------

class _EndBuildExc(Exception):
    pass


def build_kernel(P, ablate='full'):
    import concourse.bass as bass
    import concourse.bacc as bacc
    import concourse.tile as tile
    import concourse.mybir as mybir
    from concourse import library_config

    LV = {'gonly': 0, 'gred': 1, 'scr': 2, 'mlp': 3, 'noag': 4, 'full': 5}[ablate]
    dt = mybir.dt
    AF = mybir.ActivationFunctionType
    ALU = mybir.AluOpType
    AX = mybir.AxisListType
    sa, sb = P["sa"], P["sb"]

    nc = bacc.Bacc("TRN2", target_bir_lowering=False, debug=False,
                   enable_asserts=True, num_devices=NC)

    # ---- I/O -----------------------------------------------------------
    h0_d = nc.dram_tensor("h0", [TOTROWS, 128], dt.float16, kind="ExternalInput")
    ixa_d = nc.dram_tensor("idxa", [128, sa.nchunks * (GSLOT // 16)], dt.int16,
                           kind="ExternalInput")
    ixb_d = nc.dram_tensor("idxb", [128, sb.nchunks * (GSLOT // 16)], dt.int16,
                           kind="ExternalInput")
    g3a_d = nc.dram_tensor("g3a", [128, COLS // 16], dt.int16, kind="ExternalInput")
    g3b_d = nc.dram_tensor("g3b", [128, COLS // 16], dt.int16, kind="ExternalInput")
    gp_d = nc.dram_tensor("gpool", [COLS, NGRAPH], dt.float16, kind="ExternalInput")
    w1_d = nc.dram_tensor("w1", [NLAYER, 64, 64], dt.float32, kind="ExternalInput")
    w2_d = nc.dram_tensor("w2", [NLAYER, 64, 64], dt.float32, kind="ExternalInput")
    par_d = nc.dram_tensor("par", [NLAYER, 64, 6], dt.float32, kind="ExternalInput")
    l1w_d = nc.dram_tensor("l1w", [64, 64], dt.float32, kind="ExternalInput")
    l2w_d = nc.dram_tensor("l2w", [64, 10], dt.float32, kind="ExternalInput")
    lb_d = nc.dram_tensor("lb", [64, 2], dt.float32, kind="ExternalInput")
    idn_d = nc.dram_tensor("idn", [128, 128], dt.float32, kind="ExternalInput")
    out_d = nc.dram_tensor("out", [NGRAPH, 10], dt.float32, kind="ExternalOutput")

    # ---- internal DRAM -------------------------------------------------
    hstore = nc.dram_tensor("hstore", [TOTROWS, 128], dt.float16, kind="Internal")
    scr_a = nc.dram_tensor("scr_a", [sa.scr_rows, 128], dt.float16, kind="Internal")
    scr_b = nc.dram_tensor("scr_b", [sb.scr_rows, 128], dt.float16, kind="Internal")
    contrib = nc.dram_tensor("contrib", [COLS, 64], dt.float16, kind="Internal")
    agout = nc.dram_tensor("agout", [TOTROWS, 64], dt.float16, kind="Internal",
                           addr_space="Shared")
    st_in = nc.dram_tensor("st_in", [64, 2], dt.float32, kind="Internal")
    st_out = nc.dram_tensor("st_out", [64, 2], dt.float32, kind="Internal",
                            addr_space="Shared")
    pl_in = nc.dram_tensor("pl_in", [128, 64], dt.float32, kind="Internal")
    pl_out = nc.dram_tensor("pl_out", [128, 64], dt.float32, kind="Internal",
                            addr_space="Shared")

    RG = [list(range(NC))]

    import contextlib

    def _build(tc):
        nc.gpsimd.load_library(library_config.mlp)
        with tc.tile_pool(name="const", bufs=1) as constp, \
             tc.tile_pool(name="idxp", bufs=1) as idxp, \
             tc.tile_pool(name="gbuf", bufs=2) as gbufp, \
             tc.tile_pool(name="agg", bufs=1) as aggp, \
             tc.tile_pool(name="big", bufs=1) as bigp, \
             tc.tile_pool(name="stage", bufs=1) as stagep, \
             tc.tile_pool(name="wt", bufs=1) as wtp, \
             tc.tile_pool(name="mm", bufs=4, space="PSUM") as mmp, \
             tc.tile_pool(name="tp", bufs=2, space="PSUM") as tpp, \
             tc.tile_pool(name="fin", bufs=1, space="PSUM") as finp:

            # ---- one-time loads ----
            idn = constp.tile([128, 128], dt.float32, tag="idn")
            nc.sync.dma_start(idn[:], idn_d[:])
            cpack = constp.tile([64, 80], dt.float32, tag="cpack")
            nc.sync.dma_start(cpack[:, 0:64], l1w_d[:])
            nc.sync.dma_start(cpack[:, 64:74], l2w_d[:])
            nc.sync.dma_start(cpack[:, 74:76], lb_d[:])
            ixa = idxp.tile([128, sa.nchunks * (GSLOT // 16)], dt.int16, tag="ixa")
            ixb = idxp.tile([128, sb.nchunks * (GSLOT // 16)], dt.int16, tag="ixb")
            g3a = idxp.tile([128, COLS // 16], dt.int16, tag="g3a")
            g3b = idxp.tile([128, COLS // 16], dt.int16, tag="g3b")
            nc.sync.dma_start(ixa[:], ixa_d[:])
            nc.sync.dma_start(ixb[:], ixb_d[:])
            nc.sync.dma_start(g3a[:], g3a_d[:])
            nc.sync.dma_start(g3b[:], g3b_d[:])

            # zero rows of the scratch tensors
            zr = constp.tile([1, 128], dt.float16, tag="zr")
            nc.vector.memset(zr[:], 0.0)
            nc.sync.dma_start(scr_a[sa.zrow:sa.zrow + 1, :], zr[:])
            nc.sync.dma_start(scr_b[sb.zrow:sb.zrow + 1, :], zr[:])

            staging = None

            def side_pass(side, idx_tile, g3_tile, scr_d, src_rows):
                """gather+reduce+transpose+scratch+g3 for one side."""
                agg = aggp.tile([64, OWN], dt.float32, tag="agg")
                for ch in range(side.nchunks):
                    buf = gbufp.tile([128, 1, GSLOT], dt.float16, tag="gbuf")
                    nc.gpsimd.dma_gather(
                        buf[:], src_rows, idx_tile[:, ch * 512:(ch + 1) * 512],
                        GSLOT, GSLOT, 128, transpose=True,
                        single_packet=False)
                    if LV >= 1:
                        for (so, co, nwin, D) in side.runs[ch]:
                            win = buf[0:64, 0, so:so + nwin * D] \
                                .rearrange("p (n d) -> p n d", d=D)
                            nc.vector.tensor_reduce(
                                agg[:, co:co + nwin], win, axis=AX.X, op=ALU.add)
                if LV < 2:
                    return None
                sta = stagep.tile([128, NCH * 64], dt.float16, tag="scrsta")
                for t in range(NCH):
                    w = 128 if t < NCH - 1 else OWN - 128 * (NCH - 1)
                    ps = tpp.tile([128, 64], dt.float32, tag="tp")
                    nc.tensor.transpose(ps[0:w, :], agg[:, t * 128:t * 128 + w],
                                        idn[0:64, 0:64])
                    if w < 128:
                        nc.vector.memset(sta[:, t * 64:(t + 1) * 64], 0.0)
                    nc.scalar.copy(sta[0:w, t * 64:(t + 1) * 64], ps[0:w, :])
                dst = scr_d[0:side.scr_w, 0:64].rearrange(
                    "(t p) c -> p t c", p=128)
                nc.sync.dma_start(dst, sta[:].rearrange("p (t c) -> p t c", c=64))
                g3buf = gbufp.tile([128, 1, GSLOT], dt.float16, tag="gbuf")
                nc.gpsimd.dma_gather(
                    g3buf[0:128, 0:1, 0:COLS], scr_d[:], g3_tile[:],
                    COLS, COLS, 128, transpose=True, single_packet=False)
                return g3buf

            for l in range(NLAYER):
                hsrc = h0_d if l == 0 else hstore
                # consume each side's g3 buffer immediately: it lives in the
                # rotating gbuf pool, so a delayed read would deadlock the
                # next side's gathers on its slot.
                ga = side_pass(sa, ixa, g3a, scr_a, hsrc[0:R, :])
                u = bigp.tile([64, COLS], dt.float32, tag="u")
                if ga is not None:
                    nc.vector.tensor_copy(u[:], ga[0:64, 0, 0:COLS])
                gb = side_pass(sb, ixb, g3b, scr_b, hsrc[R:R + 32768, :])
                if gb is not None:
                    z = bigp.tile([64, COLS], dt.float32, tag="z")
                    nc.vector.tensor_copy(z[:], gb[0:64, 0, 0:COLS])
                    nc.vector.tensor_tensor(u[:], u[:], z[:], op=ALU.add)
                if LV < 3:
                    continue

                # weights / params for this layer
                wts = wtp.tile([64, 144], dt.float32, tag="wts")
                w1, w2, par = wts[:, 0:64], wts[:, 64:128], wts[:, 128:134]
                nc.sync.dma_start(w1, w1_d[l])
                nc.sync.dma_start(w2, w2_d[l])
                nc.sync.dma_start(par, par_d[l])

                # z = u @ W1 + b1
                z = bigp.tile([64, COLS], dt.float32, tag="z")
                for k in range(0, COLS, 512):
                    n = min(512, COLS - k)
                    ps = mmp.tile([64, 512], dt.float32, tag="mm")
                    nc.tensor.matmul(ps[:, 0:n], w1, u[:, k:k + n],
                                     start=True, stop=True)
                    nc.scalar.activation(z[:, k:k + n], ps[:, 0:n],
                                         AF.Identity, bias=par[:, 0:1])

                # BN stats (+fake-col corrections), cross-core AllReduce
                bnv = wtp.tile([64, 16], dt.float32, tag="bnv")
                s1, s2 = bnv[:, 0:1], bnv[:, 1:2]
                stp, stg = bnv[:, 2:4], bnv[:, 4:6]
                mu, var = bnv[:, 6:7], bnv[:, 7:8]
                aven, bvec = bnv[:, 8:9], bnv[:, 9:10]
                sd, rsd = bnv[:, 10:11], bnv[:, 11:12]
                nc.vector.tensor_reduce(s1, z[:], axis=AX.X, op=ALU.add)
                zsq = bigp.tile([64, COLS], dt.float32, tag="u")
                nc.scalar.activation(zsq[:], z[:], AF.Square, accum_out=s2)
                nc.vector.tensor_tensor(stp[:, 0:1], s1, par[:, 4:5], op=ALU.add)
                nc.vector.tensor_tensor(stp[:, 1:2], s2, par[:, 5:6], op=ALU.add)
                nc.sync.dma_start(st_in[:], stp)
                if LV >= 4:
                    nc.gpsimd.collective_compute(
                        "AllReduce", ALU.add, replica_groups=RG,
                        ins=[st_in[:].opt()], outs=[st_out[:].opt()])
                    nc.sync.dma_start(stg, st_out[:])
                else:
                    nc.vector.tensor_copy(stg, stp)

                # avec = gamma * rsqrt(var+eps), bvec = beta - mu*avec
                nc.vector.tensor_scalar_mul(mu, stg[:, 0:1], 1.0 / N)
                nc.vector.tensor_scalar_mul(var, stg[:, 1:2], 1.0 / N)
                nc.vector.tensor_tensor(aven, mu, mu, op=ALU.mult)
                nc.vector.tensor_tensor(var, var, aven, op=ALU.subtract)
                nc.vector.tensor_scalar_add(var, var, BN_EPS)
                nc.scalar.activation(sd, var, AF.Sqrt)
                nc.vector.reciprocal(rsd, sd)
                nc.vector.tensor_tensor(aven, par[:, 1:2], rsd, op=ALU.mult)
                nc.vector.tensor_tensor(bvec, mu, aven, op=ALU.mult)
                nc.vector.tensor_tensor(bvec, par[:, 2:3], bvec, op=ALU.subtract)

                # zb = relu(avec*z + bvec); hp = relu(zb @ W2 + b2)
                zb = bigp.tile([64, COLS], dt.float32, tag="u")
                nc.scalar.activation(zb[:], z[:], AF.Relu, bias=bvec, scale=aven)
                hp = bigp.tile([64, COLS], dt.float32, tag="z")
                for k in range(0, COLS, 512):
                    n = min(512, COLS - k)
                    ps = mmp.tile([64, 512], dt.float32, tag="mm")
                    nc.tensor.matmul(ps[:, 0:n], w2, zb[:, k:k + n],
                                     start=True, stop=True)
                    nc.scalar.activation(hp[:, k:k + n], ps[:, 0:n],
                                         AF.Relu, bias=par[:, 3:4])

                # staging: transpose+cast to fp16 rows; zero fake rows
                staging = stagep.tile([128, NCH * 64], dt.float16, tag="hsta")
                lastw = OWN - 128 * (NCH - 1)       # 106 real rows in chunk 48
                for t in range(NCH):
                    ps = tpp.tile([128, 64], dt.float32, tag="tp")
                    nc.tensor.transpose(ps[:], hp[:, t * 128:(t + 1) * 128],
                                        idn[0:64, 0:64])
                    if t < NCH - 1:
                        nc.scalar.copy(staging[:, t * 64:(t + 1) * 64], ps[:])
                    else:
                        nc.vector.memset(staging[:, t * 64:(t + 1) * 64], 0.0)
                        nc.scalar.copy(staging[0:lastw, t * 64:(t + 1) * 64],
                                       ps[0:lastw, :])

                if l < NLAYER - 1:
                    nc.sync.dma_start(
                        contrib[:].rearrange("(t p) c -> p t c", p=128),
                        staging[:].rearrange("p (t c) -> p t c", c=64))
                    if LV >= 5:
                        nc.gpsimd.collective_compute(
                            "AllGather", ALU.bypass, replica_groups=RG,
                            ins=[contrib[:].opt()], outs=[agout[:].opt()])
                        nc.sync.dma_start(hstore[:, 0:64], agout[:])
                    else:
                        for cc in range(NC):
                            nc.sync.dma_start(
                                hstore[cc * COLS:(cc + 1) * COLS, 0:64],
                                contrib[:])

            # ---- pooling + final MLP ----
            if LV < 3:
                dummy = wtp.tile([128, 16], dt.float32, tag="dummy")
                nc.vector.memset(dummy[:], 0.0)
                nc.sync.dma_start(out_d[:], dummy[:, 0:10])
                raise _EndBuildExc
            plps = finp.tile([128, 64], dt.float32, tag="plps")
            for t in range(NCH):
                gt = gbufp.tile([128, NGRAPH], dt.float16, tag="gt")
                nc.sync.dma_start(gt[:], gp_d[t * 128:(t + 1) * 128, :])
                nc.tensor.matmul(plps[:], gt[:],
                                 staging[:, t * 64:(t + 1) * 64],
                                 start=(t == 0), stop=(t == NCH - 1))
            fin = wtp.tile([128, 560], dt.float32, tag="fin")
            pls, plg = fin[:, 0:64], fin[:, 64:128]
            pcm, z1 = fin[0:64, 128:256], fin[0:64, 256:384]
            lg = fin[0:10, 384:512]
            lt, ex, res = fin[:, 512:522], fin[:, 522:532], fin[:, 537:547]
            mx, mxn = fin[:, 532:533], fin[:, 533:534]
            se, ln, lnn = fin[:, 534:535], fin[:, 535:536], fin[:, 536:537]
            nc.vector.tensor_copy(pls, plps[:])
            nc.sync.dma_start(pl_in[:], pls)
            nc.gpsimd.collective_compute(
                "AllReduce", ALU.add, replica_groups=RG,
                ins=[pl_in[:].opt()], outs=[pl_out[:].opt()])
            nc.sync.dma_start(plg, pl_out[:])
            pcmp = finp.tile([64, 128], dt.float32, tag="pcmp")
            nc.tensor.transpose(pcmp[:], plg, idn[:])
            nc.vector.tensor_copy(pcm, pcmp[:])

            z1p = finp.tile([64, 128], dt.float32, tag="pcmp")
            nc.tensor.matmul(z1p[:], cpack[:, 0:64], pcm, start=True, stop=True)
            nc.scalar.activation(z1, z1p[:], AF.Relu, bias=cpack[:, 74:75])
            lgp = finp.tile([10, 128], dt.float32, tag="pcmp")
            nc.tensor.matmul(lgp[:], cpack[:, 64:74], z1, start=True, stop=True)
            nc.scalar.activation(lg, lgp[:], AF.Identity, bias=cpack[0:10, 75:76])
            ltp = finp.tile([128, 10], dt.float32, tag="pcmp")
            nc.tensor.transpose(ltp[:], lg, idn[0:10, 0:10])
            nc.vector.tensor_copy(lt, ltp[:])

            # log_softmax rows
            nc.vector.tensor_reduce(mx, lt, axis=AX.X, op=ALU.max)
            nc.vector.tensor_scalar_mul(mxn, mx, -1.0)
            nc.scalar.activation(ex, lt, AF.Exp, bias=mxn, accum_out=se)
            nc.scalar.activation(ln, se, AF.Ln)
            nc.vector.tensor_scalar_mul(lnn, ln, -1.0)
            nc.vector.tensor_scalar_add(res, lt, mxn)
            nc.vector.tensor_scalar_add(res, res, lnn)
            nc.sync.dma_start(out_d[:], res)

    with tile.TileContext(nc) as tc:
        try:
            _build(tc)
        except _EndBuildExc:
            pass

    nc.compile()
    return nc


# ---------------------------------------------------------------------------
# entry point
# ---------------------------------------------------------------------------

def make_in_maps(P):
    sa, sb = P["sa"], P["sb"]
    maps = []
    for c in range(NC):
        maps.append({
            "h0": P["h0"],
            "idxa": wrap_idx_chunked(sa.idx[c], sa.nchunks),
            "idxb": wrap_idx_chunked(sb.idx[c], sb.nchunks),
            "g3a": wrap_idx(sa.g3[c].astype(np.int16)),
            "g3b": wrap_idx(sb.g3[c].astype(np.int16)),
            "gpool": P["Gp"][c],
            "w1": P["W1"], "w2": P["W2"], "par": P["PAR"],
            "l1w": P["lin1_W"], "l2w": P["lin2_W"], "lb": P["LINB"],
            "idn": P["IDN"],
        })
    return maps


def kernel(x, edge_index, batch, params):
    from concourse import bass_utils

    P = preprocess(x, edge_index, batch, params)
    nc = build_kernel(P)
    in_maps = make_in_maps(P)
    res = bass_utils.run_bass_kernel_spmd(nc, in_maps, core_ids=list(range(NC)))
    return res.results[0]["out"].astype(np.float32)


# revision 12
# speedup vs baseline: 1.5912x; 1.1900x over previous
"""Distributed GIN (5-layer) Bass kernel for one TRN2 chip (8 NeuronCores).

Strategy (dst-sharded message passing, replicated features):
- Each core owns 6250 dst nodes. Node features live replicated in HBM as
  fp16 rows [50176, 128] (node n at row (n//6250)*6272 + n%6250, cols 0:64).
- Per layer, each core gathers its in-edges' source rows with
  dma_gather(transpose=True) -> [128ch, slots] fp16 tiles, then does
  fixed-width windowed segment sums on the Vector engine. Window sizes are
  order statistics of per-core degree (identical across cores -> SPMD safe).
- The int16 gather-index limit (32767) forces two gather "sides": rows
  [0, 17408) and [17408, 50176). Each side reduces in its own degree-sorted
  order into a per-side scratch [rows,128] fp16 in HBM; two tiny follow-up
  gathers (g3) realign both partial aggregates to canonical column order.
- MLP: PE matmuls (64x64 weights), BatchNorm via a [64,2] AllReduce of
  sum/sumsq (+corrections for the 22 fake columns), fused scale+bias+ReLU
  on the Scalar engine. New features are cast fp16, transposed to row-major
  via PE, AllGathered (compact [6272,64]) and expanded into the row buffer.
- Final layer: global_add_pool via one-hot matmuls against the fp16
  staging, [128,64] AllReduce, tiny MLP + log_softmax on every core.
"""
import sys

sys.path.insert(0, "/opt/trn_rl_repo")

import numpy as np

N = 50000
E = 800000
NC = 8
OWN = N // NC            # 6250
COLS = 6272              # 49 * 128
NCH = COLS // 128        # 49
TOTROWS = NC * COLS      # 50176
R = 17408                # side split: rows < R -> side a, else side b
GSLOT = 4096
DIM = 64
NGRAPH = 128
NLAYER = 5
BN_EPS = 1e-5

PAD_ROW_A = OWN                     # core 0 fake row (zero), < R
PAD_ROW_B = 7 * COLS + OWN          # 50154, core 7 fake row (zero), >= R


# ---------------------------------------------------------------------------
# host-side preprocessing
# ---------------------------------------------------------------------------

class SideStruct:
    """SPMD-uniform gather/reduce structure for one side."""

    def __init__(self, per_core_rows, base, side_name):
        self.base = base
        self.name = side_name
        deg = np.zeros((NC, OWN), np.int32)
        for c in range(NC):
            for v, rr in per_core_rows[c].items():
                deg[c, v] = len(rr)
        deg = np.maximum(deg, 1)
        self.order = np.argsort(-deg, axis=1, kind="stable")
        sorted_deg = -np.sort(-deg, axis=1)
        W = sorted_deg.max(axis=0)             # [OWN] monotone non-increasing
        self.W = W
        self.buckets, self.counts = [], []
        i = 0
        while i < OWN:
            j = i
            while j < OWN and W[j] == W[i]:
                j += 1
            self.buckets.append(int(W[i]))
            self.counts.append(j - i)
            i = j
        self.cols = OWN
        self.scr_w = ((self.cols + 127) // 128) * 128      # 6272
        self.zrow = self.scr_w
        self.scr_rows = self.scr_w + 16

        # chunked slot layout: runs[chunk] = [(slot_off, col_off, nwin, D)]
        self.runs = []
        cur, slot, col = [], 0, 0
        self.nslots = 0
        for b, cnt in zip(self.buckets, self.counts):
            left = cnt
            while left > 0:
                fit = (GSLOT - slot) // b
                if fit == 0:
                    self.runs.append(cur)
                    self.nslots += GSLOT
                    cur, slot = [], 0
                    continue
                take = min(left, fit)
                cur.append((slot, col, take, b))
                slot += take * b
                col += take
                left -= take
        if cur or not self.runs:
            self.runs.append(cur)
            self.nslots += GSLOT
        self.nchunks = len(self.runs)
        assert col == self.cols

        pad_row = PAD_ROW_A if side_name == "a" else PAD_ROW_B
        self.pi_pos = np.zeros((NC, OWN), np.int32)
        self.idx = np.zeros((NC, self.nchunks * GSLOT), np.int16)
        for c in range(NC):
            win_rows = []
            for p in range(OWN):
                v = int(self.order[c, p])
                self.pi_pos[c, v] = p
                win_rows.append((per_core_rows[c].get(v, []), int(self.W[p])))
            w = 0
            flat = np.zeros(self.nchunks * GSLOT, np.int64)
            for ch, runs in enumerate(self.runs):
                base_slot = ch * GSLOT
                for (slot_off, col_off, nwin, D) in runs:
                    for k in range(nwin):
                        rows, DD = win_rows[w]
                        assert DD == D
                        s0 = base_slot + slot_off + k * D
                        nr = len(rows)
                        flat[s0:s0 + nr] = rows
                        flat[s0 + nr:s0 + D] = pad_row
                        w += 1
            assert w == self.cols
            rel = np.clip(flat - self.base, 0, None)
            assert rel.max() <= 32767
            self.idx[c] = rel.astype(np.int16)

        self.g3 = np.full((NC, COLS), self.zrow, np.int32)
        for c in range(NC):
            self.g3[c, :OWN] = self.pi_pos[c]


def wrap_idx(a):
    """flat slot order -> [128, S//16] dma_gather index layout."""
    S = len(a)
    assert S % 16 == 0
    w = a.reshape(S // 16, 16).T
    return np.ascontiguousarray(np.tile(w, (8, 1)))


def wrap_idx_chunked(a, nchunks):
    return np.hstack([wrap_idx(a[c * GSLOT:(c + 1) * GSLOT]) for c in range(nchunks)])


def preprocess(x, edge_index, batch, params):
    x = np.asarray(x, np.float32)
    src = np.asarray(edge_index[0], np.int64)
    dst = np.asarray(edge_index[1], np.int64)
    batch = np.asarray(batch, np.int64)

    srow = (src // OWN) * COLS + (src % OWN)
    dcore = dst // OWN
    dloc = dst % OWN

    rows_a = [dict() for _ in range(NC)]
    rows_b = [dict() for _ in range(NC)]
    order = np.lexsort((srow, dst))
    for e in order:
        c = int(dcore[e])
        v = int(dloc[e])
        r = int(srow[e])
        tgt = rows_a[c] if r < R else rows_b[c]
        tgt.setdefault(v, []).append(r)
    sa = SideStruct(rows_a, 0, "a")
    sb = SideStruct(rows_b, R, "b")

    h0 = np.zeros((TOTROWS, 128), np.float16)
    for c in range(NC):
        h0[c * COLS:c * COLS + OWN, :DIM] = \
            x[c * OWN:(c + 1) * OWN].astype(np.float16)

    x0cm = np.zeros((NC, 64, COLS), np.float16)
    for c in range(NC):
        x0cm[c, :, :OWN] = x[c * OWN:(c + 1) * OWN].T.astype(np.float16)

    Gp = np.zeros((NC, COLS, NGRAPH), np.float16)
    for c in range(NC):
        Gp[c, np.arange(OWN), batch[c * OWN:(c + 1) * OWN]] = 1.0

    convs = params["convs"]
    W1 = np.stack([np.asarray(p["W1"], np.float32) for p in convs])
    W2 = np.stack([np.asarray(p["W2"], np.float32) for p in convs])
    B1 = np.stack([np.asarray(p["b1"], np.float32) for p in convs])
    GM = np.stack([np.asarray(p["gamma"], np.float32) for p in convs])
    BT = np.stack([np.asarray(p["beta"], np.float32) for p in convs])
    B2 = np.stack([np.asarray(p["b2"], np.float32) for p in convs])
    nfake = COLS - OWN
    # PARAMS[l] is [64, 6] ch-major: b1, gamma, beta, b2, corr1, corr2
    PAR = np.stack([np.stack([B1[l], GM[l], BT[l], B2[l],
                              -nfake * B1[l], -nfake * B1[l] * B1[l]], axis=1)
                    for l in range(NLAYER)])                 # [5, 64, 6]
    LINB = np.zeros((64, 2), np.float32)
    LINB[:, 0] = np.asarray(params["lin1_b"], np.float32)
    LINB[:10, 1] = np.asarray(params["lin2_b"], np.float32)

    return dict(
        sa=sa, sb=sb, h0=h0, Gp=Gp, W1=W1, W2=W2, PAR=PAR, x0cm=x0cm,
        lin1_W=np.asarray(params["lin1_W"], np.float32),
        lin2_W=np.asarray(params["lin2_W"], np.float32),
        LINB=LINB, IDN=np.eye(128, dtype=np.float32),
    )


# ---------------------------------------------------------------------------
# bass kernel builder
# ---------------------------------------------------------------------------

class _EndBuildExc(Exception):
    pass


def build_kernel(P, ablate='full'):
    import concourse.bass as bass
    import concourse.bacc as bacc
    import concourse.tile as tile
    import concourse.mybir as mybir
    from concourse import library_config

    LV = {'gonly': 0, 'gred': 1, 'scr': 2, 'mlp': 3, 'noag': 4, 'full': 5}[ablate]
    dt = mybir.dt
    AF = mybir.ActivationFunctionType
    ALU = mybir.AluOpType
    AX = mybir.AxisListType
    sa, sb = P["sa"], P["sb"]

    nc = bacc.Bacc("TRN2", target_bir_lowering=False, debug=False,
                   enable_asserts=True, num_devices=NC)

    # ---- I/O -----------------------------------------------------------
    h0_d = nc.dram_tensor("h0", [TOTROWS, 128], dt.float16, kind="ExternalInput")
    ixa_d = nc.dram_tensor("idxa", [128, sa.nchunks * (GSLOT // 16)], dt.int16,
                           kind="ExternalInput")
    ixb_d = nc.dram_tensor("idxb", [128, sb.nchunks * (GSLOT // 16)], dt.int16,
                           kind="ExternalInput")
    g3a_d = nc.dram_tensor("g3a", [128, COLS // 16], dt.int16, kind="ExternalInput")
    g3b_d = nc.dram_tensor("g3b", [128, COLS // 16], dt.int16, kind="ExternalInput")
    gp_d = nc.dram_tensor("gpool", [COLS, NGRAPH], dt.float16, kind="ExternalInput")
    w1_d = nc.dram_tensor("w1", [NLAYER, 64, 64], dt.float32, kind="ExternalInput")
    w2_d = nc.dram_tensor("w2", [NLAYER, 64, 64], dt.float32, kind="ExternalInput")
    par_d = nc.dram_tensor("par", [NLAYER, 64, 6], dt.float32, kind="ExternalInput")
    l1w_d = nc.dram_tensor("l1w", [64, 64], dt.float32, kind="ExternalInput")
    l2w_d = nc.dram_tensor("l2w", [64, 10], dt.float32, kind="ExternalInput")
    lb_d = nc.dram_tensor("lb", [64, 2], dt.float32, kind="ExternalInput")
    idn_d = nc.dram_tensor("idn", [128, 128], dt.float32, kind="ExternalInput")
    x0_d = nc.dram_tensor("x0cm", [64, COLS], dt.float16, kind="ExternalInput")
    out_d = nc.dram_tensor("out", [NGRAPH, 10], dt.float32, kind="ExternalOutput")

    # ---- internal DRAM -------------------------------------------------
    hstore = nc.dram_tensor("hstore", [TOTROWS, 128], dt.float16, kind="Internal")
    scr_a = nc.dram_tensor("scr_a", [sa.scr_rows, 128], dt.float16, kind="Internal")
    scr_b = nc.dram_tensor("scr_b", [sb.scr_rows, 128], dt.float16, kind="Internal")
    contrib = nc.dram_tensor("contrib", [COLS, 64], dt.float16, kind="Internal")
    agout = nc.dram_tensor("agout", [TOTROWS, 64], dt.float16, kind="Internal",
                           addr_space="Shared")
    st_in = nc.dram_tensor("st_in", [64, 2], dt.float32, kind="Internal")
    st_out = nc.dram_tensor("st_out", [64, 2], dt.float32, kind="Internal",
                            addr_space="Shared")
    pl_in = nc.dram_tensor("pl_in", [128, 64], dt.float32, kind="Internal")
    pl_out = nc.dram_tensor("pl_out", [128, 64], dt.float32, kind="Internal",
                            addr_space="Shared")

    RG = [list(range(NC))]

    import contextlib

    def _build(tc):
        nc.gpsimd.load_library(library_config.mlp)
        with tc.tile_pool(name="const", bufs=1) as constp, \
             tc.tile_pool(name="idxp", bufs=1) as idxp, \
             tc.tile_pool(name="gbuf", bufs=2) as gbufp, \
             tc.tile_pool(name="agg", bufs=1) as aggp, \
             tc.tile_pool(name="big", bufs=1) as bigp, \
             tc.tile_pool(name="stage", bufs=2) as stagep, \
             tc.tile_pool(name="wt", bufs=1) as wtp, \
             tc.tile_pool(name="mm", bufs=4, space="PSUM") as mmp, \
             tc.tile_pool(name="tp", bufs=2, space="PSUM") as tpp, \
             tc.tile_pool(name="fin", bufs=1, space="PSUM") as finp:

            # ---- one-time loads ----
            idn = constp.tile([128, 128], dt.float32, tag="idn")
            nc.sync.dma_start(idn[:], idn_d[:])
            cpack = constp.tile([64, 80], dt.float32, tag="cpack")
            nc.sync.dma_start(cpack[:, 0:64], l1w_d[:])
            nc.sync.dma_start(cpack[:, 64:74], l2w_d[:])
            nc.sync.dma_start(cpack[:, 74:76], lb_d[:])
            ixa = idxp.tile([128, sa.nchunks * (GSLOT // 16)], dt.int16, tag="ixa")
            ixb = idxp.tile([128, sb.nchunks * (GSLOT // 16)], dt.int16, tag="ixb")
            g3a = idxp.tile([128, COLS // 16], dt.int16, tag="g3a")
            g3b = idxp.tile([128, COLS // 16], dt.int16, tag="g3b")
            nc.sync.dma_start(ixa[:], ixa_d[:])
            nc.sync.dma_start(ixb[:], ixb_d[:])
            nc.sync.dma_start(g3a[:], g3a_d[:])
            nc.sync.dma_start(g3b[:], g3b_d[:])

            # zero rows of the scratch tensors
            zr = constp.tile([1, 128], dt.float16, tag="zr")
            nc.vector.memset(zr[:], 0.0)
            nc.sync.dma_start(scr_a[sa.zrow:sa.zrow + 1, :], zr[:])
            nc.sync.dma_start(scr_b[sb.zrow:sb.zrow + 1, :], zr[:])

            # previous-layer features, ch-major fp16 (the GIN "+h" term)
            hprev = bigp.tile([64, COLS], dt.float16, tag="hprev")
            nc.sync.dma_start(hprev[:], x0_d[:])

            staging = None

            def emit_gathers(side, idx_tile, src_rows):
                """all chunk gathers + windowed reduces for one side."""
                agg = aggp.tile([64, OWN], dt.float32, tag="agg")
                for ch in range(side.nchunks):
                    buf = gbufp.tile([128, 1, GSLOT], dt.float16, tag="gbuf")
                    nc.gpsimd.dma_gather(
                        buf[:], src_rows, idx_tile[:, ch * 256:(ch + 1) * 256],
                        GSLOT, GSLOT, 128, transpose=True,
                        single_packet=False)
                    if LV >= 1:
                        for (so, co, nwin, D) in side.runs[ch]:
                            win = buf[0:64, 0, so:so + nwin * D] \
                                .rearrange("p (n d) -> p n d", d=D)
                            nc.vector.tensor_reduce(
                                agg[:, co:co + nwin], win, axis=AX.X,
                                op=ALU.add)
                return agg

            def transpose_to_rows(srcT, sta, width):
                """[64, width] f32 -> [128-row chunks x 64] fp16 staging, with
                4-chunk-batched PSUM->SBUF copies; zero rows beyond width."""
                nchf = (width + 127) // 128          # chunks incl partial
                t = 0
                while t < nchf:
                    gsz = min(4, nchf - t)
                    ps = tpp.tile([128, 256], dt.float32, tag="tp")
                    full = True
                    for g in range(gsz):
                        w = min(128, width - (t + g) * 128)
                        if w < 128:
                            full = False
                        nc.tensor.transpose(
                            ps[0:w, g * 64:g * 64 + 64],
                            srcT[:, (t + g) * 128:(t + g) * 128 + w],
                            idn[0:64, 0:64])
                    if full:
                        nc.scalar.copy(sta[:, t * 64:(t + gsz) * 64],
                                       ps[:, 0:gsz * 64])
                    else:
                        nc.vector.memset(sta[:, t * 64:(t + gsz) * 64], 0.0)
                        lw = width - (nchf - 1) * 128
                        if gsz > 1:
                            nc.scalar.copy(
                                sta[:, t * 64:(t + gsz - 1) * 64],
                                ps[:, 0:(gsz - 1) * 64])
                        nc.scalar.copy(
                            sta[0:lw, (nchf - 1) * 64:nchf * 64],
                            ps[0:lw, (gsz - 1) * 64:gsz * 64])
                    t += gsz

            def finish_side(side, agg, g3_tile, scr_d):
                """transpose agg to scratch rows, DMA out, g3 gather back."""
                sta = stagep.tile([128, 6272], dt.float16, tag="sta")
                transpose_to_rows(agg, sta[:, 0:NCH * 64], OWN)
                dst = scr_d[0:side.scr_w, 0:64].rearrange(
                    "(t p) c -> p t c", p=128)
                nc.sync.dma_start(
                    dst, sta[:, 0:NCH * 64].rearrange("p (t c) -> p t c", c=64))
                g3buf = stagep.tile([128, 6272], dt.float16, tag="sta")
                nc.gpsimd.dma_gather(
                    g3buf[:].rearrange("p (a b) -> p a b", a=1), scr_d[:],
                    g3_tile[:], COLS, COLS, 128, transpose=True,
                    single_packet=False)
                return g3buf

            for l in range(NLAYER):
                hsrc = h0_d if l == 0 else hstore
                agg_a = emit_gathers(sa, ixa, hsrc[0:R, :])
                agg_b = emit_gathers(sb, ixb, hsrc[R:R + 32768, :])
                if LV < 2:
                    continue
                ga = finish_side(sa, agg_a, g3a, scr_a)
                u = bigp.tile([64, COLS], dt.float32, tag="u")
                nc.vector.tensor_copy(u[:], ga[0:64, 0:COLS])
                gb = finish_side(sb, agg_b, g3b, scr_b)
                z = bigp.tile([64, COLS], dt.float32, tag="z")
                nc.vector.tensor_copy(z[:], gb[0:64, 0:COLS])
                nc.vector.tensor_tensor(u[:], u[:], z[:], op=ALU.add)
                nc.vector.tensor_tensor(u[:], u[:], hprev[:], op=ALU.add)
                if LV < 3:
                    continue

                # weights / params for this layer
                wts = wtp.tile([64, 144], dt.float32, tag="wts")
                w1, w2, par = wts[:, 0:64], wts[:, 64:128], wts[:, 128:134]
                nc.sync.dma_start(w1, w1_d[l])
                nc.sync.dma_start(w2, w2_d[l])
                nc.sync.dma_start(par, par_d[l])

                # z = u @ W1 + b1
                z = bigp.tile([64, COLS], dt.float32, tag="z")
                for k in range(0, COLS, 512):
                    n = min(512, COLS - k)
                    ps = mmp.tile([64, 512], dt.float32, tag="mm")
                    nc.tensor.matmul(ps[:, 0:n], w1, u[:, k:k + n],
                                     start=True, stop=True)
                    nc.scalar.activation(z[:, k:k + n], ps[:, 0:n],
                                         AF.Identity, bias=par[:, 0:1])

                # BN stats (+fake-col corrections), cross-core AllReduce
                bnv = wtp.tile([64, 16], dt.float32, tag="bnv")
                s1, s2 = bnv[:, 0:1], bnv[:, 1:2]
                stp, stg = bnv[:, 2:4], bnv[:, 4:6]
                mu, var = bnv[:, 6:7], bnv[:, 7:8]
                aven, bvec = bnv[:, 8:9], bnv[:, 9:10]
                sd, rsd = bnv[:, 10:11], bnv[:, 11:12]
                nc.vector.tensor_reduce(s1, z[:], axis=AX.X, op=ALU.add)
                zsq = bigp.tile([64, COLS], dt.float32, tag="u")
                nc.scalar.activation(zsq[:], z[:], AF.Square, accum_out=s2)
                nc.vector.tensor_tensor(stp[:, 0:1], s1, par[:, 4:5], op=ALU.add)
                nc.vector.tensor_tensor(stp[:, 1:2], s2, par[:, 5:6], op=ALU.add)
                nc.sync.dma_start(st_in[:], stp)
                if LV >= 4:
                    nc.gpsimd.collective_compute(
                        "AllReduce", ALU.add, replica_groups=RG,
                        ins=[st_in[:].opt()], outs=[st_out[:].opt()])
                    nc.sync.dma_start(stg, st_out[:])
                else:
                    nc.vector.tensor_copy(stg, stp)

                # avec = gamma * rsqrt(var+eps), bvec = beta - mu*avec
                nc.vector.tensor_scalar_mul(mu, stg[:, 0:1], 1.0 / N)
                nc.vector.tensor_scalar_mul(var, stg[:, 1:2], 1.0 / N)
                nc.vector.tensor_tensor(aven, mu, mu, op=ALU.mult)
                nc.vector.tensor_tensor(var, var, aven, op=ALU.subtract)
                nc.vector.tensor_scalar_add(var, var, BN_EPS)
                nc.scalar.activation(sd, var, AF.Sqrt)
                nc.vector.reciprocal(rsd, sd)
                nc.vector.tensor_tensor(aven, par[:, 1:2], rsd, op=ALU.mult)
                nc.vector.tensor_tensor(bvec, mu, aven, op=ALU.mult)
                nc.vector.tensor_tensor(bvec, par[:, 2:3], bvec, op=ALU.subtract)

                # zb = relu(avec*z + bvec); hp = relu(zb @ W2 + b2)
                zb = bigp.tile([64, COLS], dt.float32, tag="u")
                nc.scalar.activation(zb[:], z[:], AF.Relu, bias=bvec, scale=aven)
                hp = bigp.tile([64, COLS], dt.float32, tag="z")
                for k in range(0, COLS, 512):
                    n = min(512, COLS - k)
                    ps = mmp.tile([64, 512], dt.float32, tag="mm")
                    nc.tensor.matmul(ps[:, 0:n], w2, zb[:, k:k + n],
                                     start=True, stop=True)
                    nc.scalar.activation(hp[:, k:k + n], ps[:, 0:n],
                                         AF.Relu, bias=par[:, 3:4])

                # update hprev (fp16 cast) for the next layer's "+h"
                hprev = bigp.tile([64, COLS], dt.float16, tag="hprev")
                nc.vector.tensor_copy(hprev[:], hp[:])
                # fake cols must be exactly zero in staging rows
                nc.vector.memset(hprev[:, OWN:COLS], 0.0)

                # staging: transpose+cast to fp16 rows (fake rows zero)
                staging = stagep.tile([128, 6272], dt.float16, tag="sta")
                transpose_to_rows(hp, staging[:, 0:NCH * 64], OWN)

                if l < NLAYER - 1:
                    nc.sync.dma_start(
                        contrib[:].rearrange("(t p) c -> p t c", p=128),
                        staging[:, 0:NCH * 64].rearrange(
                            "p (t c) -> p t c", c=64))
                    if LV >= 5:
                        nc.gpsimd.collective_compute(
                            "AllGather", ALU.bypass, replica_groups=RG,
                            ins=[contrib[:].opt()], outs=[agout[:].opt()])
                        nc.sync.dma_start(hstore[:, 0:64], agout[:])
                    else:
                        for cc in range(NC):
                            nc.sync.dma_start(
                                hstore[cc * COLS:(cc + 1) * COLS, 0:64],
                                contrib[:])

            # ---- pooling + final MLP ----
            if LV < 3:
                dummy = wtp.tile([128, 16], dt.float32, tag="dummy")
                nc.vector.memset(dummy[:], 0.0)
                nc.sync.dma_start(out_d[:], dummy[:, 0:10])
                raise _EndBuildExc
            plps = finp.tile([128, 64], dt.float32, tag="plps")
            for t in range(NCH):
                gt = gbufp.tile([128, NGRAPH], dt.float16, tag="gt")
                nc.sync.dma_start(gt[:], gp_d[t * 128:(t + 1) * 128, :])
                nc.tensor.matmul(plps[:], gt[:],
                                 staging[:, t * 64:(t + 1) * 64],
                                 start=(t == 0), stop=(t == NCH - 1))
            fin = wtp.tile([128, 560], dt.float32, tag="fin")
            pls, plg = fin[:, 0:64], fin[:, 64:128]
            pcm, z1 = fin[0:64, 128:256], fin[0:64, 256:384]
            lg = fin[0:10, 384:512]
            lt, ex, res = fin[:, 512:522], fin[:, 522:532], fin[:, 537:547]
            mx, mxn = fin[:, 532:533], fin[:, 533:534]
            se, ln, lnn = fin[:, 534:535], fin[:, 535:536], fin[:, 536:537]
            nc.vector.tensor_copy(pls, plps[:])
            nc.sync.dma_start(pl_in[:], pls)
            if LV >= 4:
                nc.gpsimd.collective_compute(
                    "AllReduce", ALU.add, replica_groups=RG,
                    ins=[pl_in[:].opt()], outs=[pl_out[:].opt()])
                nc.sync.dma_start(plg, pl_out[:])
            else:
                nc.vector.tensor_copy(plg, pls)
            pcmp = finp.tile([64, 128], dt.float32, tag="pcmp")
            nc.tensor.transpose(pcmp[:], plg, idn[:])
            nc.vector.tensor_copy(pcm, pcmp[:])

            z1p = finp.tile([64, 128], dt.float32, tag="pcmp")
            nc.tensor.matmul(z1p[:], cpack[:, 0:64], pcm, start=True, stop=True)
            nc.scalar.activation(z1, z1p[:], AF.Relu, bias=cpack[:, 74:75])
            lgp = finp.tile([10, 128], dt.float32, tag="pcmp")
            nc.tensor.matmul(lgp[:], cpack[:, 64:74], z1, start=True, stop=True)
            nc.scalar.activation(lg, lgp[:], AF.Identity, bias=cpack[0:10, 75:76])
            ltp = finp.tile([128, 10], dt.float32, tag="pcmp")
            nc.tensor.transpose(ltp[:], lg, idn[0:10, 0:10])
            nc.vector.tensor_copy(lt, ltp[:])

            # log_softmax rows
            nc.vector.tensor_reduce(mx, lt, axis=AX.X, op=ALU.max)
            nc.vector.tensor_scalar_mul(mxn, mx, -1.0)
            nc.scalar.activation(ex, lt, AF.Exp, bias=mxn, accum_out=se)
            nc.scalar.activation(ln, se, AF.Ln)
            nc.vector.tensor_scalar_mul(lnn, ln, -1.0)
            nc.vector.tensor_scalar_add(res, lt, mxn)
            nc.vector.tensor_scalar_add(res, res, lnn)
            nc.sync.dma_start(out_d[:], res)

    with tile.TileContext(nc) as tc:
        try:
            _build(tc)
        except _EndBuildExc:
            pass

    nc.compile()
    return nc


# ---------------------------------------------------------------------------
# entry point
# ---------------------------------------------------------------------------

def make_in_maps(P):
    sa, sb = P["sa"], P["sb"]
    maps = []
    for c in range(NC):
        maps.append({
            "h0": P["h0"],
            "idxa": wrap_idx_chunked(sa.idx[c], sa.nchunks),
            "idxb": wrap_idx_chunked(sb.idx[c], sb.nchunks),
            "g3a": wrap_idx(sa.g3[c].astype(np.int16)),
            "g3b": wrap_idx(sb.g3[c].astype(np.int16)),
            "gpool": P["Gp"][c],
            "x0cm": P["x0cm"][c],
            "w1": P["W1"], "w2": P["W2"], "par": P["PAR"],
            "l1w": P["lin1_W"], "l2w": P["lin2_W"], "lb": P["LINB"],
            "idn": P["IDN"],
        })
    return maps


def kernel(x, edge_index, batch, params):
    from concourse import bass_utils

    P = preprocess(x, edge_index, batch, params)
    nc = build_kernel(P)
    in_maps = make_in_maps(P)
    res = bass_utils.run_bass_kernel_spmd(nc, in_maps, core_ids=list(range(NC)))
    return res.results[0]["out"].astype(np.float32)
